# revision 45
# baseline (speedup 1.0000x reference)
"""Trainium2 Bass kernel: quantized BasicBlock (quant-conv3x3 -> bn -> relu ->
quant-conv3x3 -> bn -> +residual -> relu).

Sharding: data-parallel over the batch dim of x across 8 NeuronCores (8 images
per core).  Weight quantization (centroid/deviation pipeline) is replicated on
every core, computed on-device.

Algorithm: 1-D Winograd F(2,3) along H.  Each 3x3 conv becomes, per output
half (even/odd rows), a combination of four "M" products M_pt = sum_{kw,ci}
U_pt[kw]^T @ V_pt[:, :, kw:kw+28], where V_pt are row-shift combinations of
the padded input (B^T d) and U_pt are kh-combinations of the quantized weights
(G g).  y_even = M0+M1+M2, y_odd = M1-M2-M3.  24 matmuls of N=392 per
(image, co-chunk) instead of 36 for direct conv (1.5x fewer PE cycles).

Math notes:
  - jnp.round (round-half-even) via the fp32 magic trick:
    rne(v) = (v + 1.5*2^23) - 1.5*2^23 for |v| < 2^22; fp16 variant uses
    1.5*2^10 (valid for |v| <= 2^9, used on the deviation clamp output).
  - Quantized weights are integer levels dev+cent = k/8 with |k| < 2048,
    exact in fp16.  The global scale `step` is folded into the BN scale.
  - conv2's residual is folded into PSUM with diag(1/s) fp16 matmuls.
  - Combines route M1/M2/M3 PSUM->SBUF through ScalarE (fp16) so the DVE
    adds run in 2x packed mode; only one DVE op touches PSUM directly.
"""

import sys

for _p in ("/opt/trn_rl_repo",):
    if _p not in sys.path:
        sys.path.insert(0, _p)

from contextlib import ExitStack

import numpy as np

import concourse.tile as tile
from concourse import bacc, mybir
from concourse.bass_utils import run_bass_kernel_spmd
from concourse.masks import make_identity

P = 128
B, C, H, W = 64, 256, 28, 28
NCORES = 8
BPC = B // NCORES          # images per core
CK = C // P                # channel chunks (2)
TAPS = 9
HP, WP = H + 2, W + 2      # zero-padded spatial 30x30
TY = H // 2                # winograd row-tiles per image (14)
NN = TY * W                # matmul free dim (392)
F32 = mybir.dt.float32
F16 = mybir.dt.float16

MAGIC = 12582912.0         # 1.5 * 2**23  (fp32 RNE round-to-int trick)
MAGIC16 = 1536.0           # 1.5 * 2**10  (fp16 RNE trick, |v| <= 512)
HALF_LVLS = 127.0
LV = 8.0                   # 2**(NUM_BITS-1)
CSTEP = HALF_LVLS / LV     # 15.875
DEVW = 0.5 * HALF_LVLS     # 63.5
BN_EPS = 1e-5

AF = mybir.ActivationFunctionType
OP = mybir.AluOpType
AX = mybir.AxisListType


def _emit(nc, tc, ctx, td):
    """Emit the whole per-core program.  td: dict of DRAM tensor handles."""
    const = ctx.enter_context(tc.tile_pool(name="const", bufs=1))
    bnp = ctx.enter_context(tc.tile_pool(name="bnp", bufs=2))
    wbig = ctx.enter_context(tc.tile_pool(name="wbig", bufs=1))
    whalf = ctx.enter_context(tc.tile_pool(name="whalf", bufs=1))
    wqp = ctx.enter_context(tc.tile_pool(name="wqp", bufs=1))
    wtp = ctx.enter_context(tc.tile_pool(name="wtp", bufs=1))
    wup = ctx.enter_context(tc.tile_pool(name="wup", bufs=1))
    tpp = ctx.enter_context(tc.tile_pool(name="tpp", bufs=2, space="PSUM"))
    psp = ctx.enter_context(tc.tile_pool(name="psp", bufs=6, space="PSUM"))
    pxf = ctx.enter_context(tc.tile_pool(name="pxf", bufs=2))
    pxp = ctx.enter_context(tc.tile_pool(name="pxp", bufs=7))
    pv1 = ctx.enter_context(tc.tile_pool(name="pv1", bufs=3))
    pv2 = ctx.enter_context(tc.tile_pool(name="pv2", bufs=2))
    phh = ctx.enter_context(tc.tile_pool(name="phh", bufs=3))
    pyy = ctx.enter_context(tc.tile_pool(name="pyy", bufs=2))
    pep = ctx.enter_context(tc.tile_pool(name="pep", bufs=2))

    ident16 = const.tile([P, P], F16, name="ident16", tag="ident16")
    make_identity(nc, ident16)
    ident32 = const.tile([P, P], F32, name="ident32", tag="ident32")
    make_identity(nc, ident32)
    ones32 = const.tile([1, P], F32, name="ones32", tag="ones32")
    nc.gpsimd.memset(ones32[:], 1.0)
    magicv = const.tile([P, 1], F32, name="magicv", tag="magicv")
    nc.gpsimd.memset(magicv[:], MAGIC)
    negmagicv = const.tile([P, 1], F32, name="negmagicv", tag="negmagicv")
    nc.gpsimd.memset(negmagicv[:], -MAGIC)
    # warm the ScalarE activation tables during the initial DMA wait
    scr = const.tile([P, 1], F32, name="scr", tag="scr")
    nc.scalar.activation(scr[:], magicv[:], AF.Sqrt)
    nc.scalar.activation(scr[:], magicv[:], AF.Relu)

    inv_s = {}   # BN scale with quant step folded in: [P, CK]
    bvec = {}    # BN bias: [P, CK]
    _w32 = {}
    _wq = {}
    wT = {}      # wT[j][k]: [P(ci), CK(m), TAPS, P(co)] fp16
    _u1 = {}     # _u1[j][k]: [P(ci), CK(m), 3(kw), P(co)] fp16
    _u2 = {}
    _istep = {}
    _inv = {}
    _idrs = {}   # diag(1/inv_s2) fp16 identities per m, (pos, neg)

    # ---------------- image loads ------------------------------------------
    x_view = td["x"].ap().rearrange("b (c p) h w -> b p c h w", p=P)
    y_view = td["y"].ap().rearrange("b (c p) h w -> b p c h w", p=P)
    xp_t = [None] * BPC
    h_t = [None] * BPC
    v1_t = [None] * BPC
    v2_t = [None] * BPC

    def frame_memset(t):
        nc.gpsimd.memset(t[:, :, 0, :], 0.0)
        nc.gpsimd.memset(t[:, :, HP - 1, :], 0.0)
        nc.gpsimd.memset(t[:, :, :, 0], 0.0)
        nc.gpsimd.memset(t[:, :, :, WP - 1], 0.0)

    def load_x(i):
        xf = pxf.tile([P, CK, H, W], F32, name=f"xf{i}", tag="xf")
        nc.sync.dma_start(xf[:], x_view[i])
        xp = pxp.tile([P, CK, HP, WP], F16, name=f"xp{i}", tag="xp")
        frame_memset(xp)
        nc.scalar.copy(xp[:, :, 1 : 1 + H, 1 : 1 + W], xf[:])
        xp_t[i] = xp

    # ---------------- per-weight quantization ------------------------------
    _step = {}

    def quant_dma(j):
        """Issue weight DMAs (baseline layout: partition = co-within-chunk)."""
        w32 = wbig.tile([P, CK, C, TAPS], F32, name=f"w32_{j}", tag="wbig")
        wsrc = td[f"w{j}"].ap().rearrange("(c p) ci kh kw -> p c ci (kh kw)", p=P)
        for c in range(CK):
            for k in range(CK):
                ks = slice(k * P, (k + 1) * P)
                nc.sync.dma_start(w32[:, c, ks, :], wsrc[:, c, ks, :])
        _w32[j] = w32

    def quant_absmax(j):
        """Global absmax -> step/istep (reduces on DVE, combine on PE)."""
        w32 = _w32[j]
        if j == 1:
            # latency-split: partial reduces overlap the w1 DMA chunks
            pmq = []
            for c in range(CK):
                ph = bnp.tile([P, 1], F32, name=f"pmq{j}_{c}", tag=f"pmq{c}")
                nc.vector.tensor_reduce(
                    ph[:], w32[:, c, :, :], axis=AX.XY, op=OP.max,
                    apply_absolute_value=True,
                )
                pmq.append(ph)
            pm = bnp.tile([P, 1], F32, name=f"pm{j}", tag="pm")
            nc.vector.tensor_max(pm[:], pmq[0][:], pmq[1][:])
        else:
            pm = bnp.tile([P, 1], F32, name=f"pm{j}", tag="pm")
            nc.vector.tensor_reduce(
                pm[:], w32[:], axis=AX.XYZ, op=OP.max,
                apply_absolute_value=True,
            )
        pmt = tpp.tile([1, P], F32, name=f"pmt{j}", tag="tp")
        nc.tensor.transpose(pmt[:], pm[:], ident32[:])
        sm = bnp.tile([1, 1], F32, name=f"sm{j}", tag="sm")
        nc.vector.tensor_reduce(sm[:], pmt[:], axis=AX.X, op=OP.max)
        pmb = tpp.tile([P, 1], F32, name=f"pmb{j}", tag="tp")
        nc.tensor.matmul(pmb[:], ones32[:], sm[:])
        pmax = bnp.tile([P, 1], F32, name=f"pmax{j}", tag="pmax")
        nc.vector.tensor_copy(pmax[:], pmb[:])
        step = const.tile([P, 1], F32, name=f"step{j}", tag=f"step{j}")
        nc.vector.tensor_scalar_mul(step[:], pmax[:], 1.0 / HALF_LVLS)
        _step[j] = step
        rmax = bnp.tile([P, 1], F32, name=f"rmax{j}", tag="rmax")
        nc.vector.reciprocal(rmax[:], pmax[:])
        istep = const.tile([P, 1], F32, name=f"istep{j}", tag=f"istep{j}")
        nc.vector.tensor_scalar_mul(istep[:], rmax[:], HALF_LVLS)
        _istep[j] = istep
        if j == 1:
            i16 = const.tile([P, 1], F16, name="istep16", tag="istep16")
            nc.vector.tensor_copy(i16[:], istep[:])
            _istep16[0] = i16
        # fold step into BN scale: inv_s = inv * step
        ivs = const.tile([P, CK], F32, name=f"ivs{j}", tag=f"ivs{j}")
        nc.vector.tensor_scalar_mul(ivs[:], _inv[j][:], _step[j][:, 0:1])
        inv_s[j] = ivs
        if j == 2:
            # diag(1/inv_s2) identities for the residual-in-PSUM matmuls
            rcp = bnp.tile([P, CK], F32, name="rcp2", tag="rcp2")
            nc.vector.reciprocal(rcp[:], ivs[:])
            for m in range(CK):
                idp = const.tile([P, P], F16, name=f"idrs{m}", tag=f"idrs{m}")
                nc.vector.tensor_scalar_mul(idp[:], ident16[:], rcp[:, m : m + 1])
                idn = const.tile([P, P], F16, name=f"idrsn{m}", tag=f"idrsn{m}")
                nc.vector.tensor_scalar_mul(idn[:], idp[:], -1.0)
                _idrs[m] = (idp, idn)

        wq = wqp.tile([P, CK, C, TAPS], F16, name=f"wq{j}", tag="wq")
        _wq[j] = wq
        wT[j] = []
        _u1[j] = []
        _u2[j] = []
        for k in range(CK):
            wt = wtp.tile([P, CK, TAPS, P], F16, name=f"wT{j}_{k}", tag=f"wT{j}_{k}")
            wT[j].append(wt)
            u1 = wup.tile([P, CK, 3, P], F16, name=f"u1_{j}_{k}", tag=f"u1_{j}_{k}")
            u2 = wup.tile([P, CK, 3, P], F16, name=f"u2_{j}_{k}", tag=f"u2_{j}_{k}")
            _u1[j].append(u1)
            _u2[j].append(u2)

    def bn_prep(j):
        """BN vector prep: [1,256] row loads, PE spread, math at [128, CK]."""
        gv = bnp.tile([1, C], F32, name=f"gv{j}", tag=f"gv{j}")
        bev = bnp.tile([1, C], F32, name=f"bev{j}", tag=f"bev{j}")
        muv = bnp.tile([1, C], F32, name=f"muv{j}", tag=f"muv{j}")
        vav = bnp.tile([1, C], F32, name=f"vav{j}", tag=f"vav{j}")
        nc.sync.dma_start(gv[:], td[f"gamma{j}"].ap().unsqueeze(0))
        nc.sync.dma_start(bev[:], td[f"beta{j}"].ap().unsqueeze(0))
        nc.sync.dma_start(muv[:], td[f"mean{j}"].ap().unsqueeze(0))
        nc.sync.dma_start(vav[:], td[f"var{j}"].ap().unsqueeze(0))

        psB = tpp.tile([P, 4 * CK], F32, name=f"psB{j}", tag="tp")
        for v, row in enumerate((gv, bev, muv, vav)):
            for c in range(CK):
                nc.tensor.matmul(
                    psB[:, v * CK + c : v * CK + c + 1],
                    row[0:1, c * P : (c + 1) * P],
                    ones32[0:1, 0:1],
                )
        bn4 = bnp.tile([P, 4, CK], F32, name=f"bn4_{j}", tag=f"bn4_{j}")
        nc.vector.tensor_copy(bn4[:], psB[:].rearrange("p (v c) -> p v c", c=CK))
        gvp, bevp, muvp, vavp = (bn4[:, v, :] for v in range(4))

        tv = bnp.tile([P, CK], F32, name=f"tv{j}", tag="btmp")
        nc.vector.tensor_scalar_add(tv[:], vavp, BN_EPS)
        rv = bnp.tile([P, CK], F32, name=f"rv{j}", tag="btmp")
        nc.vector.reciprocal(rv[:], tv[:])
        sv = bnp.tile([P, CK], F32, name=f"sv{j}", tag="btmp")
        nc.scalar.activation(sv[:], rv[:], AF.Sqrt)           # rsqrt(var+eps)
        inv = const.tile([P, CK], F32, name=f"inv{j}", tag=f"inv{j}")
        nc.vector.tensor_mul(inv[:], sv[:], gvp)              # gamma * rsqrt
        mi = bnp.tile([P, CK], F32, name=f"mi{j}", tag="btmp")
        nc.vector.tensor_mul(mi[:], muvp, inv[:])
        bv = const.tile([P, CK], F32, name=f"bv{j}", tag=f"bv{j}")
        nc.vector.tensor_sub(bv[:], bevp, mi[:])              # beta - mean*inv
        bvec[j] = bv
        _inv[j] = inv

    def quant_chain(j, c):
        """Quantize co-chunk c (full 256-ci row) into fp16 levels, then
        PE-transpose the taps into wT[j][k][:, c, t, :]."""
        w32, wq, istep = _w32[j], _wq[j], _istep[j]
        src = w32[:, c, :, :]                      # [P(co), 256(ci), 9]
        # wl = rne(w * istep) via fp32 magic on ScalarE
        wlr = whalf.tile([P, C, TAPS], F32, name=f"wlr{j}_{c}", tag="wh32")
        nc.scalar.activation(
            wlr[:], src, AF.Identity, bias=magicv[:, 0:1], scale=istep[:, 0:1]
        )
        wl3 = whalf.tile([P, C, TAPS], F16, name=f"wl3{j}_{c}", tag="wh16a")
        nc.vector.tensor_scalar_sub(wl3[:], wlr[:], MAGIC)

        # per-grain (co, ci) mean over the 9 taps -> centroid levels
        gm = bnp.tile([P, C], F32, name=f"gm{j}_{c}", tag="gm")
        nc.vector.tensor_reduce(gm[:], wl3[:], axis=AX.X, op=OP.add)
        c1 = bnp.tile([P, C], F32, name=f"c1{j}_{c}", tag="c1")
        nc.vector.tensor_scalar(
            c1[:], gm[:], 1.0 / (TAPS * CSTEP), MAGIC, OP.mult, OP.add
        )
        cent = bnp.tile([P, C], F16, name=f"cent{j}_{c}", tag="cent")
        nc.vector.tensor_scalar(
            cent[:], c1[:], MAGIC, CSTEP, OP.subtract, OP.mult
        )
        centb = whalf.tile([P, C, TAPS], F16, name=f"cb{j}_{c}", tag="wh16b")
        nc.vector.tensor_copy(centb[:], cent.unsqueeze(2).broadcast_to((P, C, TAPS)))

        # dev = rne(clip(wl - cent, -63.5, 63.5)); wq = dev + cent  (fp16)
        dv = whalf.tile([P, C, TAPS], F16, name=f"dv{j}_{c}", tag="wh16c")
        nc.vector.tensor_sub(dv[:], wl3[:], centb[:])
        dv2 = whalf.tile([P, C, TAPS], F16, name=f"dv2{j}_{c}", tag="wh16d")
        nc.vector.tensor_scalar(dv2[:], dv[:], DEVW, -DEVW, OP.min, OP.max)
        dv3 = whalf.tile([P, C, TAPS], F16, name=f"dv3{j}_{c}", tag="wh16a")
        nc.vector.tensor_scalar(
            dv3[:], dv2[:], MAGIC, MAGIC, OP.add, OP.subtract
        )
        nc.vector.tensor_add(wq[:, c, :, :], dv3[:], centb[:])

        # PE-transpose the 9 taps of each ci-half: [co,ci] -> [ci,co]
        for k in range(CK):
            for t0 in (0, 4, 8):
                nb = min(4, TAPS - t0)
                pst = tpp.tile([P, nb, P], F16, name=f"pst{j}_{c}_{k}_{t0}", tag="tp")
                for dt in range(nb):
                    nc.tensor.transpose(
                        pst[:, dt, :],
                        wq[:, c, k * P : (k + 1) * P, t0 + dt],
                        ident16[:],
                    )
                nc.scalar.copy(wT[j][k][:, c, t0 : t0 + nb, :], pst[:])

    def uprep(j, c):
        """Build U1/U2 fp16 slices for co-chunk c from wT (U0/U3 = wT views)."""
        for k in range(CK):
            wt = wT[j][k]
            s = pep.tile([P, 3, P], F16, name=f"us{j}_{c}_{k}", tag="us")
            nc.vector.tensor_add(s[:], wt[:, c, 0:3, :], wt[:, c, 6:9, :])
            t1 = pep.tile([P, 3, P], F16, name=f"ut1{j}_{c}_{k}", tag="us")
            nc.vector.tensor_add(t1[:], s[:], wt[:, c, 3:6, :])
            nc.vector.tensor_scalar_mul(_u1[j][k][:, c], t1[:], 0.5)
            t2 = pep.tile([P, 3, P], F16, name=f"ut2{j}_{c}_{k}", tag="us")
            nc.vector.tensor_sub(t2[:], s[:], wt[:, c, 3:6, :])
            nc.vector.tensor_scalar_mul(_u2[j][k][:, c], t2[:], 0.5)

    def ulhs(j, pt, m, k, kw):
        """lhsT [128(ci), 128(co)] for (point pt, co-half m, ci-chunk k, kw)."""
        if pt == 0:
            return wT[j][k][:, m, kw, :]
        if pt == 1:
            return _u1[j][k][:, m, kw, :]
        if pt == 2:
            return _u2[j][k][:, m, kw, :]
        return wT[j][k][:, m, 6 + kw, :]

    # ---------------- winograd input transforms ----------------------------
    def vtrans(i, src, which):
        """V_pt row-shift tiles [P, 4, CK, TY, WP] fp16 from padded src."""
        pool = pv1 if which == 1 else pv2
        vv = pool.tile([P, 4, CK, TY, WP], F16, name=f"v{which}_{i}", tag=f"v{which}")
        d0 = src[:, :, 0 : 2 * TY - 1 : 2, :]      # rows 0,2,..,26
        d1 = src[:, :, 1 : 2 * TY : 2, :]          # rows 1,3,..,27
        d2 = src[:, :, 2 : 2 * TY + 1 : 2, :]      # rows 2,4,..,28
        d3 = src[:, :, 3 : 2 * TY + 2 : 2, :]      # rows 3,5,..,29
        nc.vector.tensor_sub(vv[:, 0], d0, d2)
        nc.vector.tensor_add(vv[:, 1], d1, d2)
        nc.vector.tensor_sub(vv[:, 2], d2, d1)
        nc.gpsimd.tensor_sub(vv[:, 3], d1, d3)
        if which == 1:
            v1_t[i] = vv
        else:
            v2_t[i] = vv

    # ---------------- convolutions -----------------------------------------
    def conv_mms(i, j, m, vt, resid):
        """24 matmuls -> 4 psum tiles M0..M3 for (image i, conv j, co-half m).
        resid: fold x/s (and -x/s) into M0 / M3 via diag identities."""
        mm = []
        for pt in range(4):
            ps = psp.tile([P, NN], F32, name=f"ps{j}_{i}_{m}_{pt}", tag="ps")
            mm.append(ps)
            first = True
            if resid and pt == 0:
                nc.tensor.matmul(
                    ps[:], _idrs[m][0][:],
                    xp_t[i][:, m, 1 : 2 * TY : 2, 1 : 1 + W],
                    start=True, stop=False,
                )
                first = False
            if resid and pt == 3:
                nc.tensor.matmul(
                    ps[:], _idrs[m][1][:],
                    xp_t[i][:, m, 2 : 2 * TY + 1 : 2, 1 : 1 + W],
                    start=True, stop=False,
                )
                first = False
            idx = 0
            for k in range(CK):
                for kw in range(3):
                    idx += 1
                    nc.tensor.matmul(
                        ps[:],
                        ulhs(j, pt, m, k, kw),
                        vt[:, pt, k, :, kw : kw + W],
                        start=first and idx == 1,
                        stop=(idx == 2 * 3),
                    )
        return mm

    def combine(i, j, m, mm, dst_even, dst_odd):
        """y_even = M0+M1+M2, y_odd = M1-M2-M3 then bn+relu on ACT.
        M1/M2/M3 go PSUM->SBUF fp16 via ScalarE so DVE adds run packed."""
        b1 = pep.tile([P, NN], F16, name=f"b1_{j}_{i}_{m}", tag="b1")
        nc.scalar.copy(b1[:], mm[1][:])
        b2 = pep.tile([P, NN], F16, name=f"b2_{j}_{i}_{m}", tag="b2")
        nc.scalar.copy(b2[:], mm[2][:])
        # HAM keep-warm: tiny junk matmul gated on b1 lands here in the
        # schedule, bounding PE silence below the ~3.4us re-throttle window
        jm = tpp.tile([P, 2 * P], F32, name=f"jm{j}_{i}_{m}", tag="tp")
        nc.tensor.matmul(jm[:], ident16[:], b1[:, 0 : 2 * P])
        t1 = pep.tile([P, NN], F16, name=f"t1_{j}_{i}_{m}", tag="t1")
        nc.vector.tensor_add(t1[:], mm[0][:], b1[:])
        t2 = pep.tile([P, NN], F16, name=f"t2_{j}_{i}_{m}", tag="t2")
        nc.vector.tensor_add(t2[:], t1[:], b2[:])
        nc.scalar.activation(
            dst_even,
            t2.rearrange("p (r w) -> p r w", w=W),
            AF.Relu,
            bias=bvec[j][:, m : m + 1],
            scale=inv_s[j][:, m : m + 1],
        )
        t3 = pep.tile([P, NN], F16, name=f"t3_{j}_{i}_{m}", tag="t3")
        nc.vector.tensor_sub(t3[:], b1[:], b2[:])
        t4 = pep.tile([P, NN], F16, name=f"t4_{j}_{i}_{m}", tag="t4")
        nc.vector.tensor_sub(t4[:], t3[:], mm[3][:])
        nc.scalar.activation(
            dst_odd,
            t4.rearrange("p (r w) -> p r w", w=W),
            AF.Relu,
            bias=bvec[j][:, m : m + 1],
            scale=inv_s[j][:, m : m + 1],
        )

    def vpre(i):
        vtrans(i, xp_t[i][:], 1)

    def conv1(i, ms=None):
        if ms is None or ms == [0]:
            hh = phh.tile([P, CK, HP, WP], F16, name=f"h{i}", tag="h")
            frame_memset(hh)
            h_t[i] = hh
        hh = h_t[i]
        for m in (ms if ms is not None else range(CK)):
            mm = conv_mms(i, 1, m, v1_t[i], resid=False)
            combine(
                i, 1, m, mm,
                hh[:, m, 1 : 2 * TY : 2, 1 : 1 + W],
                hh[:, m, 2 : 2 * TY + 1 : 2, 1 : 1 + W],
            )

    def vpre2(i):
        vtrans(i, h_t[i][:], 2)

    def conv2(i):
        for m in range(CK):
            yf = pyy.tile([P, H, W], F32, name=f"y{i}_{m}", tag="y")
            mm = conv_mms(i, 2, m, v2_t[i], resid=True)
            combine(
                i, 2, m, mm,
                yf[:, 0 : 2 * TY : 2, :],
                yf[:, 1 : 2 * TY : 2, :],
            )
            nc.gpsimd.dma_start(y_view[i][:, m], yf[:])

    def pe_warmup(n, gated=False):
        """Junk matmuls to hold the PE HAM at K=8/8 through the head's DMA
        wait."""
        for i in range(n):
            scr_ps = psp.tile([P, NN], F32, name=f"warm{_wuid[0]}", tag="ps")
            _wuid[0] += 1
            if gated:
                rhsb = _istep16[0].broadcast_to((P, 3 * P))
            else:
                rhsb = ident16.unsqueeze(1).broadcast_to((P, 3, P))
            nc.tensor.matmul(scr_ps[:, 0 : 3 * P], ident16[:], rhsb)

    _wuid = [0]
    _istep16 = [None]

    # ---------------- emission order (engine priority) ---------------------
    pe_warmup(48)
    quant_dma(1)
    bn_prep(1)
    bn_prep(2)
    load_x(0)
    load_x(1)
    quant_dma(2)
    for i in range(2, BPC):
        load_x(i)
    quant_absmax(1)
    pe_warmup(32, gated=True)
    quant_chain(1, 0)
    uprep(1, 0)
    vpre(0)
    vpre(1)
    vpre(2)
    conv1(0, ms=[0])
    conv1(1, ms=[0])
    quant_chain(1, 1)
    uprep(1, 1)
    conv1(2, ms=[0])
    conv1(0, ms=[1])
    vpre(3)
    conv1(1, ms=[1])
    vpre(4)
    conv1(2, ms=[1])
    vpre(5)
    conv1(3)
    quant_absmax(2)
    quant_chain(2, 0)
    uprep(2, 0)
    vpre2(0)
    conv1(4)
    vpre(6)
    quant_chain(2, 1)
    uprep(2, 1)
    vpre2(1)
    conv1(5)
    conv2(0)
    vpre(7)
    vpre2(2)
    conv1(6)
    conv2(1)
    vpre2(3)
    conv1(7)
    vpre2(4)
    conv2(2)
    vpre2(5)
    conv2(3)
    vpre2(6)
    conv2(4)
    vpre2(7)
    for i in range(5, BPC):
        conv2(i)


def build_bass():
    nc = bacc.Bacc(
        "TRN2", target_bir_lowering=False, debug=False, num_devices=NCORES
    )
    td = {}
    td["x"] = nc.dram_tensor("x", (BPC, C, H, W), F32, kind="ExternalInput")
    for j in (1, 2):
        td[f"w{j}"] = nc.dram_tensor(f"w{j}", (C, C, 3, 3), F32, kind="ExternalInput")
        for v in ("gamma", "beta", "mean", "var"):
            td[f"{v}{j}"] = nc.dram_tensor(f"{v}{j}", (C,), F32, kind="ExternalInput")
    td["y"] = nc.dram_tensor("y", (BPC, C, H, W), F32, kind="ExternalOutput")

    with tile.TileContext(nc) as tc:
        with ExitStack() as ctx:
            _emit(nc, tc, ctx, td)
    nc.compile()
    return nc


_NC = None


def _get_nc():
    global _NC
    if _NC is None:
        _NC = build_bass()
    return _NC


def make_in_maps(x, w1, gamma1, beta1, mean1, var1, w2, gamma2, beta2, mean2, var2):
    rep = {
        "w1": w1, "gamma1": gamma1, "beta1": beta1, "mean1": mean1, "var1": var1,
        "w2": w2, "gamma2": gamma2, "beta2": beta2, "mean2": mean2, "var2": var2,
    }
    rep = {k: np.ascontiguousarray(np.asarray(v), dtype=np.float32) for k, v in rep.items()}
    in_maps = []
    for c in range(NCORES):
        m = {"x": np.ascontiguousarray(np.asarray(x)[c * BPC : (c + 1) * BPC], dtype=np.float32)}
        m.update(rep)
        in_maps.append(m)
    return in_maps


def kernel(x, w1, gamma1, beta1, mean1, var1,
           w2, gamma2, beta2, mean2, var2, codebook=None, **_unused):
    nc = _get_nc()
    in_maps = make_in_maps(x, w1, gamma1, beta1, mean1, var1,
                           w2, gamma2, beta2, mean2, var2)
    res = run_bass_kernel_spmd(nc, in_maps, core_ids=list(range(NCORES)))
    return np.concatenate([r["y"] for r in res.results], axis=0)


# revision 46
# speedup vs baseline: 1.0116x; 1.0116x over previous
"""Trainium2 Bass kernel: quantized BasicBlock (quant-conv3x3 -> bn -> relu ->
quant-conv3x3 -> bn -> +residual -> relu).

Sharding: data-parallel over the batch dim of x across 8 NeuronCores (8 images
per core).  Weight quantization (centroid/deviation pipeline) is replicated on
every core, computed on-device.

Algorithm: 1-D Winograd F(2,3) along H.  Each 3x3 conv becomes, per output
half (even/odd rows), a combination of four "M" products M_pt = sum_{kw,ci}
U_pt[kw]^T @ V_pt[:, :, kw:kw+28], where V_pt are row-shift combinations of
the padded input (B^T d) and U_pt are kh-combinations of the quantized weights
(G g).  y_even = M0+M1+M2, y_odd = M1-M2-M3.  24 matmuls of N=392 per
(image, co-chunk) instead of 36 for direct conv (1.5x fewer PE cycles).

Math notes:
  - jnp.round (round-half-even) via the fp32 magic trick:
    rne(v) = (v + 1.5*2^23) - 1.5*2^23 for |v| < 2^22; fp16 variant uses
    1.5*2^10 (valid for |v| <= 2^9, used on the deviation clamp output).
  - Quantized weights are integer levels dev+cent = k/8 with |k| < 2048,
    exact in fp16.  The global scale `step` is folded into the BN scale.
  - conv2's residual is folded into PSUM with diag(1/s) fp16 matmuls.
  - Combines route M1/M2/M3 PSUM->SBUF through ScalarE (fp16) so the DVE
    adds run in 2x packed mode; only one DVE op touches PSUM directly.
"""

import sys

for _p in ("/opt/trn_rl_repo",):
    if _p not in sys.path:
        sys.path.insert(0, _p)

from contextlib import ExitStack

import numpy as np

import concourse.tile as tile
from concourse import bacc, mybir
from concourse.bass_utils import run_bass_kernel_spmd
from concourse.masks import make_identity

P = 128
B, C, H, W = 64, 256, 28, 28
NCORES = 8
BPC = B // NCORES          # images per core
CK = C // P                # channel chunks (2)
TAPS = 9
HP, WP = H + 2, W + 2      # zero-padded spatial 30x30
TY = H // 2                # winograd row-tiles per image (14)
NN = TY * W                # matmul free dim (392)
F32 = mybir.dt.float32
F16 = mybir.dt.float16

MAGIC = 12582912.0         # 1.5 * 2**23  (fp32 RNE round-to-int trick)
MAGIC16 = 1536.0           # 1.5 * 2**10  (fp16 RNE trick, |v| <= 512)
HALF_LVLS = 127.0
LV = 8.0                   # 2**(NUM_BITS-1)
CSTEP = HALF_LVLS / LV     # 15.875
DEVW = 0.5 * HALF_LVLS     # 63.5
BN_EPS = 1e-5

AF = mybir.ActivationFunctionType
OP = mybir.AluOpType
AX = mybir.AxisListType


def _emit(nc, tc, ctx, td):
    """Emit the whole per-core program.  td: dict of DRAM tensor handles."""
    const = ctx.enter_context(tc.tile_pool(name="const", bufs=1))
    bnp = ctx.enter_context(tc.tile_pool(name="bnp", bufs=2))
    wbig = ctx.enter_context(tc.tile_pool(name="wbig", bufs=1))
    whalf = ctx.enter_context(tc.tile_pool(name="whalf", bufs=1))
    wqp = ctx.enter_context(tc.tile_pool(name="wqp", bufs=1))
    wtp = ctx.enter_context(tc.tile_pool(name="wtp", bufs=1))
    wup = ctx.enter_context(tc.tile_pool(name="wup", bufs=1))
    tpp = ctx.enter_context(tc.tile_pool(name="tpp", bufs=2, space="PSUM"))
    psp = ctx.enter_context(tc.tile_pool(name="psp", bufs=6, space="PSUM"))
    pxf = ctx.enter_context(tc.tile_pool(name="pxf", bufs=2))
    pxp = ctx.enter_context(tc.tile_pool(name="pxp", bufs=7))
    pv1 = ctx.enter_context(tc.tile_pool(name="pv1", bufs=3))
    pv2 = ctx.enter_context(tc.tile_pool(name="pv2", bufs=2))
    phh = ctx.enter_context(tc.tile_pool(name="phh", bufs=3))
    pyy = ctx.enter_context(tc.tile_pool(name="pyy", bufs=1))
    pep = ctx.enter_context(tc.tile_pool(name="pep", bufs=2))

    ident16 = const.tile([P, P], F16, name="ident16", tag="ident16")
    make_identity(nc, ident16)
    ident32 = const.tile([P, P], F32, name="ident32", tag="ident32")
    make_identity(nc, ident32)
    ones32 = const.tile([1, P], F32, name="ones32", tag="ones32")
    nc.gpsimd.memset(ones32[:], 1.0)
    magicv = const.tile([P, 1], F32, name="magicv", tag="magicv")
    nc.gpsimd.memset(magicv[:], MAGIC)
    negmagicv = const.tile([P, 1], F32, name="negmagicv", tag="negmagicv")
    nc.gpsimd.memset(negmagicv[:], -MAGIC)
    # warm the ScalarE activation tables during the initial DMA wait
    scr = const.tile([P, 1], F32, name="scr", tag="scr")
    nc.scalar.activation(scr[:], magicv[:], AF.Sqrt)
    nc.scalar.activation(scr[:], magicv[:], AF.Relu)

    inv_s = {}   # BN scale with quant step folded in: [P, CK]
    bvec = {}    # BN bias: [P, CK]
    _w32 = {}
    _wq = {}
    wT = {}      # wT[j][k]: [P(ci), CK(m), TAPS, P(co)] fp16
    _u1 = {}     # _u1[j][k]: [P(ci), CK(m), 3(kw), P(co)] fp16
    _u2 = {}
    _istep = {}
    _inv = {}
    _idrs = {}   # diag(1/inv_s2) fp16 identities per m, (pos, neg)

    # ---------------- image loads ------------------------------------------
    x_view = td["x"].ap().rearrange("b (c p) h w -> b p c h w", p=P)
    y_view = td["y"].ap().rearrange("b (c p) h w -> b p c h w", p=P)
    xp_t = [None] * BPC
    h_t = [None] * BPC
    v1_t = [None] * BPC
    v2_t = [None] * BPC

    def frame_memset(t):
        nc.gpsimd.memset(t[:, :, 0, :], 0.0)
        nc.gpsimd.memset(t[:, :, HP - 1, :], 0.0)
        nc.gpsimd.memset(t[:, :, :, 0], 0.0)
        nc.gpsimd.memset(t[:, :, :, WP - 1], 0.0)

    def load_x(i):
        xf = pxf.tile([P, CK, H, W], F32, name=f"xf{i}", tag="xf")
        nc.sync.dma_start(xf[:], x_view[i])
        xp = pxp.tile([P, CK, HP, WP], F16, name=f"xp{i}", tag="xp")
        frame_memset(xp)
        nc.scalar.copy(xp[:, :, 1 : 1 + H, 1 : 1 + W], xf[:])
        xp_t[i] = xp

    # ---------------- per-weight quantization ------------------------------
    _step = {}

    def quant_dma(j):
        """Issue weight DMAs (baseline layout: partition = co-within-chunk)."""
        w32 = wbig.tile([P, CK, C, TAPS], F32, name=f"w32_{j}", tag="wbig")
        wsrc = td[f"w{j}"].ap().rearrange("(c p) ci kh kw -> p c ci (kh kw)", p=P)
        for c in range(CK):
            for k in range(CK):
                ks = slice(k * P, (k + 1) * P)
                nc.sync.dma_start(w32[:, c, ks, :], wsrc[:, c, ks, :])
        _w32[j] = w32

    def quant_absmax(j):
        """Global absmax -> step/istep (reduces on DVE, combine on PE)."""
        w32 = _w32[j]
        if j == 1:
            # latency-split: partial reduces overlap the w1 DMA chunks
            pmq = []
            for c in range(CK):
                ph = bnp.tile([P, 1], F32, name=f"pmq{j}_{c}", tag=f"pmq{c}")
                nc.vector.tensor_reduce(
                    ph[:], w32[:, c, :, :], axis=AX.XY, op=OP.max,
                    apply_absolute_value=True,
                )
                pmq.append(ph)
            pm = bnp.tile([P, 1], F32, name=f"pm{j}", tag="pm")
            nc.vector.tensor_max(pm[:], pmq[0][:], pmq[1][:])
        else:
            pm = bnp.tile([P, 1], F32, name=f"pm{j}", tag="pm")
            nc.vector.tensor_reduce(
                pm[:], w32[:], axis=AX.XYZ, op=OP.max,
                apply_absolute_value=True,
            )
        pmt = tpp.tile([1, P], F32, name=f"pmt{j}", tag="tp")
        nc.tensor.transpose(pmt[:], pm[:], ident32[:])
        sm = bnp.tile([1, 1], F32, name=f"sm{j}", tag="sm")
        nc.vector.tensor_reduce(sm[:], pmt[:], axis=AX.X, op=OP.max)
        pmb = tpp.tile([P, 1], F32, name=f"pmb{j}", tag="tp")
        nc.tensor.matmul(pmb[:], ones32[:], sm[:])
        pmax = bnp.tile([P, 1], F32, name=f"pmax{j}", tag="pmax")
        nc.vector.tensor_copy(pmax[:], pmb[:])
        step = const.tile([P, 1], F32, name=f"step{j}", tag=f"step{j}")
        nc.vector.tensor_scalar_mul(step[:], pmax[:], 1.0 / HALF_LVLS)
        _step[j] = step
        rmax = bnp.tile([P, 1], F32, name=f"rmax{j}", tag="rmax")
        nc.vector.reciprocal(rmax[:], pmax[:])
        istep = const.tile([P, 1], F32, name=f"istep{j}", tag=f"istep{j}")
        nc.vector.tensor_scalar_mul(istep[:], rmax[:], HALF_LVLS)
        _istep[j] = istep
        if j == 1:
            i16 = const.tile([P, 1], F16, name="istep16", tag="istep16")
            nc.vector.tensor_copy(i16[:], istep[:])
            _istep16[0] = i16
        # fold step into BN scale: inv_s = inv * step
        ivs = const.tile([P, CK], F32, name=f"ivs{j}", tag=f"ivs{j}")
        nc.vector.tensor_scalar_mul(ivs[:], _inv[j][:], _step[j][:, 0:1])
        inv_s[j] = ivs
        if j == 2:
            # diag(1/inv_s2) identities for the residual-in-PSUM matmuls
            rcp = bnp.tile([P, CK], F32, name="rcp2", tag="rcp2")
            nc.vector.reciprocal(rcp[:], ivs[:])
            for m in range(CK):
                idp = const.tile([P, P], F16, name=f"idrs{m}", tag=f"idrs{m}")
                nc.vector.tensor_scalar_mul(idp[:], ident16[:], rcp[:, m : m + 1])
                idn = const.tile([P, P], F16, name=f"idrsn{m}", tag=f"idrsn{m}")
                nc.vector.tensor_scalar_mul(idn[:], idp[:], -1.0)
                _idrs[m] = (idp, idn)

        wq = wqp.tile([P, CK, C, TAPS], F16, name=f"wq{j}", tag="wq")
        _wq[j] = wq
        wT[j] = []
        _u1[j] = []
        _u2[j] = []
        for k in range(CK):
            wt = wtp.tile([P, CK, TAPS, P], F16, name=f"wT{j}_{k}", tag=f"wT{j}_{k}")
            wT[j].append(wt)
            u1 = wup.tile([P, CK, 3, P], F16, name=f"u1_{j}_{k}", tag=f"u1_{j}_{k}")
            u2 = wup.tile([P, CK, 3, P], F16, name=f"u2_{j}_{k}", tag=f"u2_{j}_{k}")
            _u1[j].append(u1)
            _u2[j].append(u2)

    def bn_prep(j):
        """BN vector prep: [1,256] row loads, PE spread, math at [128, CK]."""
        gv = bnp.tile([1, C], F32, name=f"gv{j}", tag=f"gv{j}")
        bev = bnp.tile([1, C], F32, name=f"bev{j}", tag=f"bev{j}")
        muv = bnp.tile([1, C], F32, name=f"muv{j}", tag=f"muv{j}")
        vav = bnp.tile([1, C], F32, name=f"vav{j}", tag=f"vav{j}")
        nc.sync.dma_start(gv[:], td[f"gamma{j}"].ap().unsqueeze(0))
        nc.sync.dma_start(bev[:], td[f"beta{j}"].ap().unsqueeze(0))
        nc.sync.dma_start(muv[:], td[f"mean{j}"].ap().unsqueeze(0))
        nc.sync.dma_start(vav[:], td[f"var{j}"].ap().unsqueeze(0))

        psB = tpp.tile([P, 4 * CK], F32, name=f"psB{j}", tag="tp")
        for v, row in enumerate((gv, bev, muv, vav)):
            for c in range(CK):
                nc.tensor.matmul(
                    psB[:, v * CK + c : v * CK + c + 1],
                    row[0:1, c * P : (c + 1) * P],
                    ones32[0:1, 0:1],
                )
        bn4 = bnp.tile([P, 4, CK], F32, name=f"bn4_{j}", tag=f"bn4_{j}")
        nc.vector.tensor_copy(bn4[:], psB[:].rearrange("p (v c) -> p v c", c=CK))
        gvp, bevp, muvp, vavp = (bn4[:, v, :] for v in range(4))

        tv = bnp.tile([P, CK], F32, name=f"tv{j}", tag="btmp")
        nc.vector.tensor_scalar_add(tv[:], vavp, BN_EPS)
        rv = bnp.tile([P, CK], F32, name=f"rv{j}", tag="btmp")
        nc.vector.reciprocal(rv[:], tv[:])
        sv = bnp.tile([P, CK], F32, name=f"sv{j}", tag="btmp")
        nc.scalar.activation(sv[:], rv[:], AF.Sqrt)           # rsqrt(var+eps)
        inv = const.tile([P, CK], F32, name=f"inv{j}", tag=f"inv{j}")
        nc.vector.tensor_mul(inv[:], sv[:], gvp)              # gamma * rsqrt
        mi = bnp.tile([P, CK], F32, name=f"mi{j}", tag="btmp")
        nc.vector.tensor_mul(mi[:], muvp, inv[:])
        bv = const.tile([P, CK], F32, name=f"bv{j}", tag=f"bv{j}")
        nc.vector.tensor_sub(bv[:], bevp, mi[:])              # beta - mean*inv
        bvec[j] = bv
        _inv[j] = inv

    def quant_chain(j, c):
        """Quantize co-chunk c (full 256-ci row) into fp16 levels, then
        PE-transpose the taps into wT[j][k][:, c, t, :]."""
        w32, wq, istep = _w32[j], _wq[j], _istep[j]
        src = w32[:, c, :, :]                      # [P(co), 256(ci), 9]
        # wl = rne(w * istep) via fp32 magic on ScalarE
        wlr = whalf.tile([P, C, TAPS], F32, name=f"wlr{j}_{c}", tag="wh32")
        nc.scalar.activation(
            wlr[:], src, AF.Identity, bias=magicv[:, 0:1], scale=istep[:, 0:1]
        )
        wl3 = whalf.tile([P, C, TAPS], F16, name=f"wl3{j}_{c}", tag="wh16a")
        nc.vector.tensor_scalar_sub(wl3[:], wlr[:], MAGIC)

        # per-grain (co, ci) mean over the 9 taps -> centroid levels
        gm = bnp.tile([P, C], F32, name=f"gm{j}_{c}", tag="gm")
        nc.vector.tensor_reduce(gm[:], wl3[:], axis=AX.X, op=OP.add)
        c1 = bnp.tile([P, C], F32, name=f"c1{j}_{c}", tag="c1")
        nc.vector.tensor_scalar(
            c1[:], gm[:], 1.0 / (TAPS * CSTEP), MAGIC, OP.mult, OP.add
        )
        cent = bnp.tile([P, C], F16, name=f"cent{j}_{c}", tag="cent")
        nc.vector.tensor_scalar(
            cent[:], c1[:], MAGIC, CSTEP, OP.subtract, OP.mult
        )
        centb = whalf.tile([P, C, TAPS], F16, name=f"cb{j}_{c}", tag="wh16b")
        nc.vector.tensor_copy(centb[:], cent.unsqueeze(2).broadcast_to((P, C, TAPS)))

        # dev = rne(clip(wl - cent, -63.5, 63.5)); wq = dev + cent  (fp16)
        dv = whalf.tile([P, C, TAPS], F16, name=f"dv{j}_{c}", tag="wh16c")
        nc.vector.tensor_sub(dv[:], wl3[:], centb[:])
        dv2 = whalf.tile([P, C, TAPS], F16, name=f"dv2{j}_{c}", tag="wh16d")
        nc.vector.tensor_scalar(dv2[:], dv[:], DEVW, -DEVW, OP.min, OP.max)
        dv3 = whalf.tile([P, C, TAPS], F16, name=f"dv3{j}_{c}", tag="wh16a")
        nc.vector.tensor_scalar(
            dv3[:], dv2[:], MAGIC, MAGIC, OP.add, OP.subtract
        )
        nc.vector.tensor_add(wq[:, c, :, :], dv3[:], centb[:])

        # PE-transpose the 9 taps of each ci-half: [co,ci] -> [ci,co]
        for k in range(CK):
            for t0 in (0, 4, 8):
                nb = min(4, TAPS - t0)
                pst = tpp.tile([P, nb, P], F16, name=f"pst{j}_{c}_{k}_{t0}", tag="tp")
                for dt in range(nb):
                    nc.tensor.transpose(
                        pst[:, dt, :],
                        wq[:, c, k * P : (k + 1) * P, t0 + dt],
                        ident16[:],
                    )
                nc.scalar.copy(wT[j][k][:, c, t0 : t0 + nb, :], pst[:])

    def uprep(j, c):
        """Build U1/U2 fp16 slices for co-chunk c from wT (U0/U3 = wT views)."""
        for k in range(CK):
            wt = wT[j][k]
            s = pep.tile([P, 3, P], F16, name=f"us{j}_{c}_{k}", tag="us")
            nc.vector.tensor_add(s[:], wt[:, c, 0:3, :], wt[:, c, 6:9, :])
            t1 = pep.tile([P, 3, P], F16, name=f"ut1{j}_{c}_{k}", tag="us")
            nc.vector.tensor_add(t1[:], s[:], wt[:, c, 3:6, :])
            nc.vector.tensor_scalar_mul(_u1[j][k][:, c], t1[:], 0.5)
            t2 = pep.tile([P, 3, P], F16, name=f"ut2{j}_{c}_{k}", tag="us")
            nc.vector.tensor_sub(t2[:], s[:], wt[:, c, 3:6, :])
            nc.vector.tensor_scalar_mul(_u2[j][k][:, c], t2[:], 0.5)

    def ulhs(j, pt, m, k, kw):
        """lhsT [128(ci), 128(co)] for (point pt, co-half m, ci-chunk k, kw)."""
        if pt == 0:
            return wT[j][k][:, m, kw, :]
        if pt == 1:
            return _u1[j][k][:, m, kw, :]
        if pt == 2:
            return _u2[j][k][:, m, kw, :]
        return wT[j][k][:, m, 6 + kw, :]

    # ---------------- winograd input transforms ----------------------------
    def vtrans(i, src, which):
        """V_pt row-shift tiles [P, 4, CK, TY, WP] fp16 from padded src."""
        pool = pv1 if which == 1 else pv2
        vv = pool.tile([P, 4, CK, TY, WP], F16, name=f"v{which}_{i}", tag=f"v{which}")
        d0 = src[:, :, 0 : 2 * TY - 1 : 2, :]      # rows 0,2,..,26
        d1 = src[:, :, 1 : 2 * TY : 2, :]          # rows 1,3,..,27
        d2 = src[:, :, 2 : 2 * TY + 1 : 2, :]      # rows 2,4,..,28
        d3 = src[:, :, 3 : 2 * TY + 2 : 2, :]      # rows 3,5,..,29
        nc.vector.tensor_sub(vv[:, 0], d0, d2)
        nc.vector.tensor_add(vv[:, 1], d1, d2)
        nc.vector.tensor_sub(vv[:, 2], d2, d1)
        nc.gpsimd.tensor_sub(vv[:, 3], d1, d3)
        if which == 1:
            v1_t[i] = vv
        else:
            v2_t[i] = vv

    # ---------------- convolutions -----------------------------------------
    def conv_mms(i, j, m, vt, resid):
        """24 matmuls -> 4 psum tiles M0..M3 for (image i, conv j, co-half m).
        resid: fold x/s (and -x/s) into M0 / M3 via diag identities."""
        mm = []
        for pt in range(4):
            ps = psp.tile([P, NN], F32, name=f"ps{j}_{i}_{m}_{pt}", tag="ps")
            mm.append(ps)
            first = True
            if resid and pt == 0:
                nc.tensor.matmul(
                    ps[:], _idrs[m][0][:],
                    xp_t[i][:, m, 1 : 2 * TY : 2, 1 : 1 + W],
                    start=True, stop=False,
                )
                first = False
            if resid and pt == 3:
                nc.tensor.matmul(
                    ps[:], _idrs[m][1][:],
                    xp_t[i][:, m, 2 : 2 * TY + 1 : 2, 1 : 1 + W],
                    start=True, stop=False,
                )
                first = False
            idx = 0
            for k in range(CK):
                for kw in range(3):
                    idx += 1
                    nc.tensor.matmul(
                        ps[:],
                        ulhs(j, pt, m, k, kw),
                        vt[:, pt, k, :, kw : kw + W],
                        start=first and idx == 1,
                        stop=(idx == 2 * 3),
                    )
        return mm

    def combine(i, j, m, mm, dst_even, dst_odd):
        """y_even = M0+M1+M2, y_odd = M1-M2-M3 then bn+relu on ACT.
        M1/M2/M3 go PSUM->SBUF fp16 via ScalarE so DVE adds run packed."""
        b1 = pep.tile([P, NN], F16, name=f"b1_{j}_{i}_{m}", tag="b1")
        nc.scalar.copy(b1[:], mm[1][:])
        b2 = pep.tile([P, NN], F16, name=f"b2_{j}_{i}_{m}", tag="b2")
        nc.scalar.copy(b2[:], mm[2][:])
        t1 = pep.tile([P, NN], F16, name=f"t1_{j}_{i}_{m}", tag="t1")
        nc.vector.tensor_add(t1[:], mm[0][:], b1[:])
        t2 = pep.tile([P, NN], F16, name=f"t2_{j}_{i}_{m}", tag="t2")
        nc.vector.tensor_add(t2[:], t1[:], b2[:])
        nc.scalar.activation(
            dst_even,
            t2.rearrange("p (r w) -> p r w", w=W),
            AF.Relu,
            bias=bvec[j][:, m : m + 1],
            scale=inv_s[j][:, m : m + 1],
        )
        t3 = pep.tile([P, NN], F16, name=f"t3_{j}_{i}_{m}", tag="t3")
        nc.vector.tensor_sub(t3[:], b1[:], b2[:])
        t4 = pep.tile([P, NN], F16, name=f"t4_{j}_{i}_{m}", tag="t4")
        nc.vector.tensor_sub(t4[:], t3[:], mm[3][:])
        nc.scalar.activation(
            dst_odd,
            t4.rearrange("p (r w) -> p r w", w=W),
            AF.Relu,
            bias=bvec[j][:, m : m + 1],
            scale=inv_s[j][:, m : m + 1],
        )

    def vpre(i):
        vtrans(i, xp_t[i][:], 1)

    def conv1(i, ms=None):
        if ms is None or ms == [0]:
            hh = phh.tile([P, CK, HP, WP], F16, name=f"h{i}", tag="h")
            frame_memset(hh)
            h_t[i] = hh
        hh = h_t[i]
        for m in (ms if ms is not None else range(CK)):
            mm = conv_mms(i, 1, m, v1_t[i], resid=False)
            combine(
                i, 1, m, mm,
                hh[:, m, 1 : 2 * TY : 2, 1 : 1 + W],
                hh[:, m, 2 : 2 * TY + 1 : 2, 1 : 1 + W],
            )

    def vpre2(i):
        vtrans(i, h_t[i][:], 2)

    def conv2(i):
        yf = pyy.tile([P, CK, H, W], F32, name=f"y{i}", tag="y")
        for m in range(CK):
            mm = conv_mms(i, 2, m, v2_t[i], resid=True)
            combine(
                i, 2, m, mm,
                yf[:, m, 0 : 2 * TY : 2, :],
                yf[:, m, 1 : 2 * TY : 2, :],
            )
            nc.gpsimd.dma_start(y_view[i][:, m], yf[:, m])

    def pe_warmup(n, gated=False):
        """Junk matmuls to hold the PE HAM at K=8/8 through the head's DMA
        wait."""
        for i in range(n):
            scr_ps = psp.tile([P, NN], F32, name=f"warm{_wuid[0]}", tag="ps")
            _wuid[0] += 1
            if gated:
                rhsb = _istep16[0].broadcast_to((P, 3 * P))
            else:
                rhsb = ident16.unsqueeze(1).broadcast_to((P, 3, P))
            nc.tensor.matmul(scr_ps[:, 0 : 3 * P], ident16[:], rhsb)

    _wuid = [0]
    _istep16 = [None]

    # ---------------- emission order (engine priority) ---------------------
    pe_warmup(48)
    quant_dma(1)
    bn_prep(1)
    bn_prep(2)
    load_x(0)
    load_x(1)
    quant_dma(2)
    for i in range(2, BPC):
        load_x(i)
    quant_absmax(1)
    pe_warmup(32, gated=True)
    quant_chain(1, 0)
    uprep(1, 0)
    vpre(0)
    vpre(1)
    vpre(2)
    conv1(0, ms=[0])
    conv1(1, ms=[0])
    quant_chain(1, 1)
    uprep(1, 1)
    conv1(2, ms=[0])
    conv1(0, ms=[1])
    vpre(3)
    conv1(1, ms=[1])
    vpre(4)
    conv1(2, ms=[1])
    vpre(5)
    conv1(3)
    quant_absmax(2)
    quant_chain(2, 0)
    uprep(2, 0)
    vpre2(0)
    conv1(4)
    vpre(6)
    quant_chain(2, 1)
    uprep(2, 1)
    vpre2(1)
    conv1(5)
    conv2(0)
    vpre(7)
    vpre2(2)
    conv1(6)
    conv2(1)
    vpre2(3)
    conv1(7)
    vpre2(4)
    conv2(2)
    vpre2(5)
    conv2(3)
    vpre2(6)
    conv2(4)
    vpre2(7)
    for i in range(5, BPC):
        conv2(i)


def build_bass():
    nc = bacc.Bacc(
        "TRN2", target_bir_lowering=False, debug=False, num_devices=NCORES
    )
    td = {}
    td["x"] = nc.dram_tensor("x", (BPC, C, H, W), F32, kind="ExternalInput")
    for j in (1, 2):
        td[f"w{j}"] = nc.dram_tensor(f"w{j}", (C, C, 3, 3), F32, kind="ExternalInput")
        for v in ("gamma", "beta", "mean", "var"):
            td[f"{v}{j}"] = nc.dram_tensor(f"{v}{j}", (C,), F32, kind="ExternalInput")
    td["y"] = nc.dram_tensor("y", (BPC, C, H, W), F32, kind="ExternalOutput")

    with tile.TileContext(nc) as tc:
        with ExitStack() as ctx:
            _emit(nc, tc, ctx, td)
    nc.compile()
    return nc


_NC = None


def _get_nc():
    global _NC
    if _NC is None:
        _NC = build_bass()
    return _NC


def make_in_maps(x, w1, gamma1, beta1, mean1, var1, w2, gamma2, beta2, mean2, var2):
    rep = {
        "w1": w1, "gamma1": gamma1, "beta1": beta1, "mean1": mean1, "var1": var1,
        "w2": w2, "gamma2": gamma2, "beta2": beta2, "mean2": mean2, "var2": var2,
    }
    rep = {k: np.ascontiguousarray(np.asarray(v), dtype=np.float32) for k, v in rep.items()}
    in_maps = []
    for c in range(NCORES):
        m = {"x": np.ascontiguousarray(np.asarray(x)[c * BPC : (c + 1) * BPC], dtype=np.float32)}
        m.update(rep)
        in_maps.append(m)
    return in_maps


def kernel(x, w1, gamma1, beta1, mean1, var1,
           w2, gamma2, beta2, mean2, var2, codebook=None, **_unused):
    nc = _get_nc()
    in_maps = make_in_maps(x, w1, gamma1, beta1, mean1, var1,
                           w2, gamma2, beta2, mean2, var2)
    res = run_bass_kernel_spmd(nc, in_maps, core_ids=list(range(NCORES)))
    return np.concatenate([r["y"] for r in res.results], axis=0)


# revision 48
# speedup vs baseline: 1.1963x; 1.1826x over previous
"""Trainium2 Bass kernel: quantized BasicBlock (quant-conv3x3 -> bn -> relu ->
quant-conv3x3 -> bn -> +residual -> relu).

Sharding: data-parallel over the batch dim of x across 8 NeuronCores (8 images
per core).  Weight quantization (centroid/deviation pipeline) is replicated on
every core, computed on-device.

Algorithm: 1-D Winograd F(2,3) along H.  Each 3x3 conv becomes, per output
half (even/odd rows), a combination of four "M" products M_pt = sum_{kw,ci}
U_pt[kw]^T @ V_pt[:, :, kw:kw+28], where V_pt are row-shift combinations of
the padded input (B^T d) and U_pt are kh-combinations of the quantized weights
(G g).  y_even = M0+M1+M2, y_odd = M1-M2-M3.  24 matmuls of N=392 per
(image, co-chunk) instead of 36 for direct conv (1.5x fewer PE cycles).

Math notes:
  - jnp.round (round-half-even) via the fp32 magic trick:
    rne(v) = (v + 1.5*2^23) - 1.5*2^23 for |v| < 2^22; fp16 variant uses
    1.5*2^10 (valid for |v| <= 2^9, used on the deviation clamp output).
  - Quantized weights are integer levels dev+cent = k/8 with |k| < 2048,
    exact in fp16.  The global scale `step` is folded into the BN scale.
  - conv2's residual is folded into PSUM with diag(1/s) fp16 matmuls.
  - Combines route M1/M2/M3 PSUM->SBUF through ScalarE (fp16) so the DVE
    adds run in 2x packed mode; only one DVE op touches PSUM directly.
"""

import sys

for _p in ("/opt/trn_rl_repo",):
    if _p not in sys.path:
        sys.path.insert(0, _p)

from contextlib import ExitStack

import numpy as np

import concourse.tile as tile
from concourse import bacc, mybir
from concourse.bass_utils import run_bass_kernel_spmd
from concourse.masks import make_identity

P = 128
B, C, H, W = 64, 256, 28, 28
NCORES = 8
BPC = B // NCORES          # images per core
CK = C // P                # channel chunks (2)
TAPS = 9
HP, WP = H + 2, W + 2      # zero-padded spatial 30x30
TY = H // 2                # winograd row-tiles per image (14)
NN = TY * W                # matmul free dim (392)
F32 = mybir.dt.float32
F16 = mybir.dt.float16

MAGIC = 12582912.0         # 1.5 * 2**23  (fp32 RNE round-to-int trick)
MAGIC16 = 1536.0           # 1.5 * 2**10  (fp16 RNE trick, |v| <= 512)
HALF_LVLS = 127.0
LV = 8.0                   # 2**(NUM_BITS-1)
CSTEP = HALF_LVLS / LV     # 15.875
DEVW = 0.5 * HALF_LVLS     # 63.5
BN_EPS = 1e-5

AF = mybir.ActivationFunctionType
OP = mybir.AluOpType
AX = mybir.AxisListType


def _emit(nc, tc, ctx, td):
    """Emit the whole per-core program.  td: dict of DRAM tensor handles."""
    const = ctx.enter_context(tc.tile_pool(name="const", bufs=1))
    bnp = ctx.enter_context(tc.tile_pool(name="bnp", bufs=2))
    wbig = ctx.enter_context(tc.tile_pool(name="wbig", bufs=1))
    whalf = ctx.enter_context(tc.tile_pool(name="whalf", bufs=1))
    wqp = ctx.enter_context(tc.tile_pool(name="wqp", bufs=1))
    wtp = ctx.enter_context(tc.tile_pool(name="wtp", bufs=1))
    wup = ctx.enter_context(tc.tile_pool(name="wup", bufs=1))
    tpp = ctx.enter_context(tc.tile_pool(name="tpp", bufs=2, space="PSUM"))
    psp = ctx.enter_context(tc.tile_pool(name="psp", bufs=6, space="PSUM"))
    pxf = ctx.enter_context(tc.tile_pool(name="pxf", bufs=2))
    pxp = ctx.enter_context(tc.tile_pool(name="pxp", bufs=6))
    pv1 = ctx.enter_context(tc.tile_pool(name="pv1", bufs=3))
    pv2 = ctx.enter_context(tc.tile_pool(name="pv2", bufs=2))
    phh = ctx.enter_context(tc.tile_pool(name="phh", bufs=3))
    pyy = ctx.enter_context(tc.tile_pool(name="pyy", bufs=2))
    pep = ctx.enter_context(tc.tile_pool(name="pep", bufs=2))

    ident16 = const.tile([P, P], F16, name="ident16", tag="ident16")
    make_identity(nc, ident16)
    ident32 = const.tile([P, P], F32, name="ident32", tag="ident32")
    make_identity(nc, ident32)
    ones32 = const.tile([1, P], F32, name="ones32", tag="ones32")
    nc.gpsimd.memset(ones32[:], 1.0)
    magicv = const.tile([P, 1], F32, name="magicv", tag="magicv")
    nc.gpsimd.memset(magicv[:], MAGIC)
    negmagicv = const.tile([P, 1], F32, name="negmagicv", tag="negmagicv")
    nc.gpsimd.memset(negmagicv[:], -MAGIC)
    # warm the ScalarE activation tables during the initial DMA wait
    scr = const.tile([P, 1], F32, name="scr", tag="scr")
    nc.scalar.activation(scr[:], magicv[:], AF.Sqrt)
    nc.scalar.activation(scr[:], magicv[:], AF.Relu)

    inv_s = {}   # BN scale with quant step folded in: [P, CK]
    bvec = {}    # BN bias: [P, CK]
    _w32 = {}
    _wq = {}
    wT = {}      # wT[j][k]: [P(ci), CK(m), TAPS, P(co)] fp16
    _u1 = {}     # _u1[j][k]: [P(ci), CK(m), 3(kw), P(co)] fp16
    _u2 = {}
    _istep = {}
    _inv = {}
    _idrs = {}   # diag(1/inv_s2) fp16 identities per m, (pos, neg)

    # ---------------- image loads ------------------------------------------
    x_view = td["x"].ap().rearrange("b (c p) h w -> b p c h w", p=P)
    y_view = td["y"].ap().rearrange("b (c p) h w -> b p c h w", p=P)
    xp_t = [None] * BPC
    h_t = [None] * BPC
    v1_t = [None] * BPC
    v2_t = [None] * BPC

    def frame_memset(t):
        nc.gpsimd.memset(t[:, :, 0, :], 0.0)
        nc.gpsimd.memset(t[:, :, HP - 1, :], 0.0)
        nc.gpsimd.memset(t[:, :, :, 0], 0.0)
        nc.gpsimd.memset(t[:, :, :, WP - 1], 0.0)

    def load_x(i):
        xf = pxf.tile([P, CK, H, W], F32, name=f"xf{i}", tag="xf")
        nc.sync.dma_start(xf[:], x_view[i])
        xp = pxp.tile([P, CK, HP, WP], F16, name=f"xp{i}", tag="xp")
        frame_memset(xp)
        nc.scalar.copy(xp[:, :, 1 : 1 + H, 1 : 1 + W], xf[:])
        xp_t[i] = xp

    # ---------------- per-weight quantization ------------------------------
    _step = {}

    def quant_dma(j):
        """Issue weight DMAs (baseline layout: partition = co-within-chunk)."""
        w32 = wbig.tile([P, CK, C, TAPS], F32, name=f"w32_{j}", tag="wbig")
        wsrc = td[f"w{j}"].ap().rearrange("(c p) ci kh kw -> p c ci (kh kw)", p=P)
        for c in range(CK):
            for k in range(CK):
                ks = slice(k * P, (k + 1) * P)
                nc.sync.dma_start(w32[:, c, ks, :], wsrc[:, c, ks, :])
        _w32[j] = w32

    def quant_absmax(j):
        """Global absmax -> step/istep (reduces on DVE, combine on PE)."""
        w32 = _w32[j]
        if j == 1:
            # latency-split: partial reduces overlap the w1 DMA chunks
            pmq = []
            for c in range(CK):
                ph = bnp.tile([P, 1], F32, name=f"pmq{j}_{c}", tag=f"pmq{c}")
                nc.vector.tensor_reduce(
                    ph[:], w32[:, c, :, :], axis=AX.XY, op=OP.max,
                    apply_absolute_value=True,
                )
                pmq.append(ph)
            pm = bnp.tile([P, 1], F32, name=f"pm{j}", tag="pm")
            nc.vector.tensor_max(pm[:], pmq[0][:], pmq[1][:])
        else:
            pm = bnp.tile([P, 1], F32, name=f"pm{j}", tag="pm")
            nc.vector.tensor_reduce(
                pm[:], w32[:], axis=AX.XYZ, op=OP.max,
                apply_absolute_value=True,
            )
        pmt = tpp.tile([1, P], F32, name=f"pmt{j}", tag="tp")
        nc.tensor.transpose(pmt[:], pm[:], ident32[:])
        sm = bnp.tile([1, 1], F32, name=f"sm{j}", tag="sm")
        nc.vector.tensor_reduce(sm[:], pmt[:], axis=AX.X, op=OP.max)
        pmb = tpp.tile([P, 1], F32, name=f"pmb{j}", tag="tp")
        nc.tensor.matmul(pmb[:], ones32[:], sm[:])
        pmax = bnp.tile([P, 1], F32, name=f"pmax{j}", tag="pmax")
        nc.vector.tensor_copy(pmax[:], pmb[:])
        step = const.tile([P, 1], F32, name=f"step{j}", tag=f"step{j}")
        nc.vector.tensor_scalar_mul(step[:], pmax[:], 1.0 / HALF_LVLS)
        _step[j] = step
        rmax = bnp.tile([P, 1], F32, name=f"rmax{j}", tag="rmax")
        nc.vector.reciprocal(rmax[:], pmax[:])
        istep = const.tile([P, 1], F32, name=f"istep{j}", tag=f"istep{j}")
        nc.vector.tensor_scalar_mul(istep[:], rmax[:], HALF_LVLS)
        _istep[j] = istep
        if j == 1:
            i16 = const.tile([P, 1], F16, name="istep16", tag="istep16")
            nc.vector.tensor_copy(i16[:], istep[:])
            _istep16[0] = i16
        # fold step into BN scale: inv_s = inv * step
        ivs = const.tile([P, CK], F32, name=f"ivs{j}", tag=f"ivs{j}")
        nc.vector.tensor_scalar_mul(ivs[:], _inv[j][:], _step[j][:, 0:1])
        inv_s[j] = ivs
        if j == 2:
            # diag(1/inv_s2) identities for the residual-in-PSUM matmuls
            rcp = bnp.tile([P, CK], F32, name="rcp2", tag="rcp2")
            nc.vector.reciprocal(rcp[:], ivs[:])
            for m in range(CK):
                idp = const.tile([P, P], F16, name=f"idrs{m}", tag=f"idrs{m}")
                nc.vector.tensor_scalar_mul(idp[:], ident16[:], rcp[:, m : m + 1])
                idn = const.tile([P, P], F16, name=f"idrsn{m}", tag=f"idrsn{m}")
                nc.vector.tensor_scalar_mul(idn[:], idp[:], -1.0)
                _idrs[m] = (idp, idn)

        wq = wqp.tile([P, CK, C, TAPS], F16, name=f"wq{j}", tag="wq")
        _wq[j] = wq
        wT[j] = []
        _u1[j] = []
        _u2[j] = []
        for k in range(CK):
            wt = wtp.tile([P, CK, TAPS, P], F16, name=f"wT{j}_{k}", tag=f"wT{j}_{k}")
            wT[j].append(wt)
            u1 = wup.tile([P, CK, 3, P], F16, name=f"u1_{j}_{k}", tag=f"u1_{j}_{k}")
            u2 = wup.tile([P, CK, 3, P], F16, name=f"u2_{j}_{k}", tag=f"u2_{j}_{k}")
            _u1[j].append(u1)
            _u2[j].append(u2)

    def bn_prep(j):
        """BN vector prep: [1,256] row loads, PE spread, math at [128, CK]."""
        gv = bnp.tile([1, C], F32, name=f"gv{j}", tag=f"gv{j}")
        bev = bnp.tile([1, C], F32, name=f"bev{j}", tag=f"bev{j}")
        muv = bnp.tile([1, C], F32, name=f"muv{j}", tag=f"muv{j}")
        vav = bnp.tile([1, C], F32, name=f"vav{j}", tag=f"vav{j}")
        nc.sync.dma_start(gv[:], td[f"gamma{j}"].ap().unsqueeze(0))
        nc.sync.dma_start(bev[:], td[f"beta{j}"].ap().unsqueeze(0))
        nc.sync.dma_start(muv[:], td[f"mean{j}"].ap().unsqueeze(0))
        nc.sync.dma_start(vav[:], td[f"var{j}"].ap().unsqueeze(0))

        psB = tpp.tile([P, 4 * CK], F32, name=f"psB{j}", tag="tp")
        for v, row in enumerate((gv, bev, muv, vav)):
            for c in range(CK):
                nc.tensor.matmul(
                    psB[:, v * CK + c : v * CK + c + 1],
                    row[0:1, c * P : (c + 1) * P],
                    ones32[0:1, 0:1],
                )
        bn4 = bnp.tile([P, 4, CK], F32, name=f"bn4_{j}", tag=f"bn4_{j}")
        nc.vector.tensor_copy(bn4[:], psB[:].rearrange("p (v c) -> p v c", c=CK))
        gvp, bevp, muvp, vavp = (bn4[:, v, :] for v in range(4))

        tv = bnp.tile([P, CK], F32, name=f"tv{j}", tag="btmp")
        nc.vector.tensor_scalar_add(tv[:], vavp, BN_EPS)
        rv = bnp.tile([P, CK], F32, name=f"rv{j}", tag="btmp")
        nc.vector.reciprocal(rv[:], tv[:])
        sv = bnp.tile([P, CK], F32, name=f"sv{j}", tag="btmp")
        nc.scalar.activation(sv[:], rv[:], AF.Sqrt)           # rsqrt(var+eps)
        inv = const.tile([P, CK], F32, name=f"inv{j}", tag=f"inv{j}")
        nc.vector.tensor_mul(inv[:], sv[:], gvp)              # gamma * rsqrt
        mi = bnp.tile([P, CK], F32, name=f"mi{j}", tag="btmp")
        nc.vector.tensor_mul(mi[:], muvp, inv[:])
        bv = const.tile([P, CK], F32, name=f"bv{j}", tag=f"bv{j}")
        nc.vector.tensor_sub(bv[:], bevp, mi[:])              # beta - mean*inv
        bvec[j] = bv
        _inv[j] = inv

    def quant_chain(j, c):
        """Quantize co-chunk c (full 256-ci row) into fp16 levels, then
        PE-transpose the taps into wT[j][k][:, c, t, :]."""
        w32, wq, istep = _w32[j], _wq[j], _istep[j]
        src = w32[:, c, :, :]                      # [P(co), 256(ci), 9]
        # wl = rne(w * istep) via fp32 magic on ScalarE
        wlr = whalf.tile([P, C, TAPS], F32, name=f"wlr{j}_{c}", tag="wh32")
        nc.scalar.activation(
            wlr[:], src, AF.Identity, bias=magicv[:, 0:1], scale=istep[:, 0:1]
        )
        wl3 = whalf.tile([P, C, TAPS], F16, name=f"wl3{j}_{c}", tag="wh16a")
        nc.vector.tensor_scalar_sub(wl3[:], wlr[:], MAGIC)

        # per-grain (co, ci) mean over the 9 taps -> centroid levels
        gm = bnp.tile([P, C], F16, name=f"gm{j}_{c}", tag="gm")
        with nc.allow_low_precision(reason="tap-sum of integer levels <= 1143, exact in fp16"):
            nc.vector.tensor_reduce(gm[:], wl3[:], axis=AX.X, op=OP.add)
        c1 = bnp.tile([P, C], F32, name=f"c1{j}_{c}", tag="c1")
        nc.vector.tensor_scalar(
            c1[:], gm[:], 1.0 / (TAPS * CSTEP), MAGIC, OP.mult, OP.add
        )
        cent = bnp.tile([P, C], F16, name=f"cent{j}_{c}", tag="cent")
        nc.vector.tensor_scalar(
            cent[:], c1[:], MAGIC, CSTEP, OP.subtract, OP.mult
        )
        centb = whalf.tile([P, C, TAPS], F16, name=f"cb{j}_{c}", tag="wh16b")
        nc.vector.tensor_copy(centb[:], cent.unsqueeze(2).broadcast_to((P, C, TAPS)))

        # dev = rne(clip(wl - cent, -63.5, 63.5)); wq = dev + cent  (fp16)
        dv = whalf.tile([P, C, TAPS], F16, name=f"dv{j}_{c}", tag="wh16c")
        nc.vector.tensor_sub(dv[:], wl3[:], centb[:])
        dv2 = whalf.tile([P, C, TAPS], F16, name=f"dv2{j}_{c}", tag="wh16d")
        nc.vector.tensor_scalar(dv2[:], dv[:], DEVW, -DEVW, OP.min, OP.max)
        dv3 = whalf.tile([P, C, TAPS], F16, name=f"dv3{j}_{c}", tag="wh16a")
        nc.vector.tensor_scalar(
            dv3[:], dv2[:], MAGIC, MAGIC, OP.add, OP.subtract
        )
        nc.vector.tensor_add(wq[:, c, :, :], dv3[:], centb[:])

        # PE-transpose the 9 taps of each ci-half: [co,ci] -> [ci,co]
        for k in range(CK):
            for t0 in (0, 4, 8):
                nb = min(4, TAPS - t0)
                pst = tpp.tile([P, nb, P], F16, name=f"pst{j}_{c}_{k}_{t0}", tag="tp")
                for dt in range(nb):
                    nc.tensor.transpose(
                        pst[:, dt, :],
                        wq[:, c, k * P : (k + 1) * P, t0 + dt],
                        ident16[:],
                    )
                nc.scalar.copy(wT[j][k][:, c, t0 : t0 + nb, :], pst[:])

    def uprep(j, c):
        """Build U1/U2 fp16 slices for co-chunk c from wT (U0/U3 = wT views)."""
        for k in range(CK):
            wt = wT[j][k]
            s = pep.tile([P, 3, P], F16, name=f"us{j}_{c}_{k}", tag="us")
            nc.vector.tensor_add(s[:], wt[:, c, 0:3, :], wt[:, c, 6:9, :])
            t1 = pep.tile([P, 3, P], F16, name=f"ut1{j}_{c}_{k}", tag="us")
            nc.vector.tensor_add(t1[:], s[:], wt[:, c, 3:6, :])
            nc.vector.tensor_scalar_mul(_u1[j][k][:, c], t1[:], 0.5)
            t2 = pep.tile([P, 3, P], F16, name=f"ut2{j}_{c}_{k}", tag="us")
            nc.vector.tensor_sub(t2[:], s[:], wt[:, c, 3:6, :])
            nc.vector.tensor_scalar_mul(_u2[j][k][:, c], t2[:], 0.5)

    def ulhs(j, pt, m, k, kw):
        """lhsT [128(ci), 128(co)] for (point pt, co-half m, ci-chunk k, kw)."""
        if pt == 0:
            return wT[j][k][:, m, kw, :]
        if pt == 1:
            return _u1[j][k][:, m, kw, :]
        if pt == 2:
            return _u2[j][k][:, m, kw, :]
        return wT[j][k][:, m, 6 + kw, :]

    # ---------------- winograd input transforms ----------------------------
    def vtrans(i, src, which):
        """V_pt row-shift tiles [P, 4, CK, TY, WP] fp16 from padded src."""
        pool = pv1 if which == 1 else pv2
        vv = pool.tile([P, 4, CK, TY, WP], F16, name=f"v{which}_{i}", tag=f"v{which}")
        d0 = src[:, :, 0 : 2 * TY - 1 : 2, :]      # rows 0,2,..,26
        d1 = src[:, :, 1 : 2 * TY : 2, :]          # rows 1,3,..,27
        d2 = src[:, :, 2 : 2 * TY + 1 : 2, :]      # rows 2,4,..,28
        d3 = src[:, :, 3 : 2 * TY + 2 : 2, :]      # rows 3,5,..,29
        nc.vector.tensor_sub(vv[:, 0], d0, d2)
        nc.vector.tensor_add(vv[:, 1], d1, d2)
        nc.vector.tensor_sub(vv[:, 2], d2, d1)
        nc.gpsimd.tensor_sub(vv[:, 3], d1, d3)
        if which == 1:
            v1_t[i] = vv
        else:
            v2_t[i] = vv

    # ---------------- convolutions -----------------------------------------
    def conv_mms(i, j, m, vt, resid):
        """24 matmuls -> 4 psum tiles M0..M3 for (image i, conv j, co-half m).
        resid: fold x/s (and -x/s) into M0 / M3 via diag identities."""
        mm = []
        for pt in range(4):
            ps = psp.tile([P, NN], F32, name=f"ps{j}_{i}_{m}_{pt}", tag="ps")
            mm.append(ps)
            first = True
            if resid and pt == 0:
                nc.tensor.matmul(
                    ps[:], _idrs[m][0][:],
                    xp_t[i][:, m, 1 : 2 * TY : 2, 1 : 1 + W],
                    start=True, stop=False,
                )
                first = False
            if resid and pt == 3:
                nc.tensor.matmul(
                    ps[:], _idrs[m][1][:],
                    xp_t[i][:, m, 2 : 2 * TY + 1 : 2, 1 : 1 + W],
                    start=True, stop=False,
                )
                first = False
            idx = 0
            for k in range(CK):
                for kw in range(3):
                    idx += 1
                    nc.tensor.matmul(
                        ps[:],
                        ulhs(j, pt, m, k, kw),
                        vt[:, pt, k, :, kw : kw + W],
                        start=first and idx == 1,
                        stop=(idx == 2 * 3),
                    )
        return mm

    def combine(i, j, m, mm, dst_even, dst_odd):
        """y_even = M0+M1+M2, y_odd = M1-M2-M3 then bn+relu on ACT.
        M1/M2/M3 go PSUM->SBUF fp16 via ScalarE so DVE adds run packed."""
        b1 = pep.tile([P, NN], F16, name=f"b1_{j}_{i}_{m}", tag="b1")
        nc.scalar.copy(b1[:], mm[1][:])
        b2 = pep.tile([P, NN], F16, name=f"b2_{j}_{i}_{m}", tag="b2")
        nc.scalar.copy(b2[:], mm[2][:])
        t1 = pep.tile([P, NN], F16, name=f"t1_{j}_{i}_{m}", tag="t1")
        nc.vector.tensor_add(t1[:], mm[0][:], b1[:])
        t2 = pep.tile([P, NN], F16, name=f"t2_{j}_{i}_{m}", tag="t2")
        nc.vector.tensor_add(t2[:], t1[:], b2[:])
        nc.scalar.activation(
            dst_even,
            t2.rearrange("p (r w) -> p r w", w=W),
            AF.Relu,
            bias=bvec[j][:, m : m + 1],
            scale=inv_s[j][:, m : m + 1],
        )
        t3 = pep.tile([P, NN], F16, name=f"t3_{j}_{i}_{m}", tag="t3")
        nc.vector.tensor_sub(t3[:], b1[:], b2[:])
        t4 = pep.tile([P, NN], F16, name=f"t4_{j}_{i}_{m}", tag="t4")
        nc.vector.tensor_sub(t4[:], t3[:], mm[3][:])
        nc.scalar.activation(
            dst_odd,
            t4.rearrange("p (r w) -> p r w", w=W),
            AF.Relu,
            bias=bvec[j][:, m : m + 1],
            scale=inv_s[j][:, m : m + 1],
        )

    def vpre(i):
        vtrans(i, xp_t[i][:], 1)

    def conv1(i, ms=None):
        if ms is None or ms == [0]:
            hh = phh.tile([P, CK, HP, WP], F16, name=f"h{i}", tag="h")
            frame_memset(hh)
            h_t[i] = hh
        hh = h_t[i]
        for m in (ms if ms is not None else range(CK)):
            mm = conv_mms(i, 1, m, v1_t[i], resid=False)
            combine(
                i, 1, m, mm,
                hh[:, m, 1 : 2 * TY : 2, 1 : 1 + W],
                hh[:, m, 2 : 2 * TY + 1 : 2, 1 : 1 + W],
            )

    def vpre2(i):
        vtrans(i, h_t[i][:], 2)

    def conv2(i):
        yf = pyy.tile([P, CK, H, W], F32, name=f"y{i}", tag="y")
        for m in range(CK):
            mm = conv_mms(i, 2, m, v2_t[i], resid=True)
            combine(
                i, 2, m, mm,
                yf[:, m, 0 : 2 * TY : 2, :],
                yf[:, m, 1 : 2 * TY : 2, :],
            )
            nc.gpsimd.dma_start(y_view[i][:, m], yf[:, m])

    def pe_warmup(n, gated=False):
        """Junk matmuls to hold the PE HAM at K=8/8 through the head's DMA
        wait."""
        for i in range(n):
            scr_ps = psp.tile([P, NN], F32, name=f"warm{_wuid[0]}", tag="ps")
            _wuid[0] += 1
            if gated:
                rhsb = _istep16[0].broadcast_to((P, 3 * P))
            else:
                rhsb = ident16.unsqueeze(1).broadcast_to((P, 3, P))
            nc.tensor.matmul(scr_ps[:, 0 : 3 * P], ident16[:], rhsb)

    _wuid = [0]
    _istep16 = [None]

    # ---------------- emission order (engine priority) ---------------------
    pe_warmup(48)
    quant_dma(1)
    bn_prep(1)
    bn_prep(2)
    load_x(0)
    load_x(1)
    quant_dma(2)
    for i in range(2, BPC):
        load_x(i)
    quant_absmax(1)
    pe_warmup(32, gated=True)
    quant_chain(1, 0)
    uprep(1, 0)
    vpre(0)
    vpre(1)
    vpre(2)
    conv1(0, ms=[0])
    conv1(1, ms=[0])
    quant_chain(1, 1)
    uprep(1, 1)
    conv1(2, ms=[0])
    conv1(0, ms=[1])
    vpre(3)
    conv1(1, ms=[1])
    vpre(4)
    conv1(2, ms=[1])
    vpre(5)
    conv1(3)
    quant_absmax(2)
    quant_chain(2, 0)
    uprep(2, 0)
    vpre2(0)
    conv1(4)
    vpre(6)
    quant_chain(2, 1)
    uprep(2, 1)
    vpre2(1)
    conv1(5)
    conv2(0)
    vpre(7)
    vpre2(2)
    conv1(6)
    conv2(1)
    vpre2(3)
    conv1(7)
    vpre2(4)
    conv2(2)
    vpre2(5)
    conv2(3)
    vpre2(6)
    conv2(4)
    vpre2(7)
    for i in range(5, BPC):
        conv2(i)


def build_bass():
    nc = bacc.Bacc(
        "TRN2", target_bir_lowering=False, debug=False, num_devices=NCORES
    )
    td = {}
    td["x"] = nc.dram_tensor("x", (BPC, C, H, W), F32, kind="ExternalInput")
    for j in (1, 2):
        td[f"w{j}"] = nc.dram_tensor(f"w{j}", (C, C, 3, 3), F32, kind="ExternalInput")
        for v in ("gamma", "beta", "mean", "var"):
            td[f"{v}{j}"] = nc.dram_tensor(f"{v}{j}", (C,), F32, kind="ExternalInput")
    td["y"] = nc.dram_tensor("y", (BPC, C, H, W), F32, kind="ExternalOutput")

    with tile.TileContext(nc) as tc:
        with ExitStack() as ctx:
            _emit(nc, tc, ctx, td)
    nc.compile()
    return nc


_NC = None


def _get_nc():
    global _NC
    if _NC is None:
        _NC = build_bass()
    return _NC


def make_in_maps(x, w1, gamma1, beta1, mean1, var1, w2, gamma2, beta2, mean2, var2):
    rep = {
        "w1": w1, "gamma1": gamma1, "beta1": beta1, "mean1": mean1, "var1": var1,
        "w2": w2, "gamma2": gamma2, "beta2": beta2, "mean2": mean2, "var2": var2,
    }
    rep = {k: np.ascontiguousarray(np.asarray(v), dtype=np.float32) for k, v in rep.items()}
    in_maps = []
    for c in range(NCORES):
        m = {"x": np.ascontiguousarray(np.asarray(x)[c * BPC : (c + 1) * BPC], dtype=np.float32)}
        m.update(rep)
        in_maps.append(m)
    return in_maps


def kernel(x, w1, gamma1, beta1, mean1, var1,
           w2, gamma2, beta2, mean2, var2, codebook=None, **_unused):
    nc = _get_nc()
    in_maps = make_in_maps(x, w1, gamma1, beta1, mean1, var1,
                           w2, gamma2, beta2, mean2, var2)
    res = run_bass_kernel_spmd(nc, in_maps, core_ids=list(range(NCORES)))
    return np.concatenate([r["y"] for r in res.results], axis=0)
